# revision 9
# baseline (speedup 1.0000x reference)
"""Trainium2 Bass kernel for nn_DeformNet (3x conv-BN blocks + deformable conv
block + avgpool + linear), data-parallel over 8 NeuronCores.

Per-core algorithm (shard of 256 images, everything in [C, (b, h, w)] layout,
fp16 compute / f32 PSUM + statistics):
  conv1  im2col K=27 matmul                  -> relu -> global BN (allreduce)
  conv2  x-im2col K=96, 3 y-tap matmuls      -> relu -> global BN
  conv3  9-tap K=64 matmuls                  -> relu -> global BN
  offset conv 9-tap K=128 -> px/py -> bilinear weight matrix
     M^T[(g,i,j), (b,hw,n)] = relu(1-|px-i|) * relu(1-|py-j|)   (i,j in 0..3;
     out-of-range samples only touch zero padding, so interior positions
     suffice and no clipping corrections are needed)
  sampling: per 8-image chunk  x_off = transpose(x3_chunk) matmul'd with M^T,
     then 9 tap matmuls with Wd  -> relu -> global BN
  mean over hw fused with BN4 affine -> linear (f32)

BN is training-mode over the full 2048-image batch: per-core sum/sumsq are
AllReduce'd across the 8 cores (exact parity with the reference).
"""

import sys

sys.path.insert(0, "/opt/trn_rl_repo")

import numpy as np

import concourse.bass as bass
import concourse.mybir as mybir
from concourse import tile
from concourse.bass_utils import run_bass_kernel_spmd

F16 = mybir.dt.float16
F32 = mybir.dt.float32
AL = mybir.AluOpType
AF = mybir.ActivationFunctionType

NCORES = 8
B_FULL = 2048
BS = B_FULL // NCORES        # 256 images per core
HW = 16
COLS = BS * HW               # 4096
NCH = BS // 8                # 32 chunks of 8 images
SCH = 8 * HW * 9             # 1152 sample-cols per chunk
S = NCH * SCH                # 36864 samples
NQ = 4                       # px_rep staged in 4 quarters (SBUF budget)
CHQ = NCH // NQ              # 8 chunks per quarter
CNT = float(B_FULL * HW)     # BN element count per channel
EPS = 1e-5
BIG = 1.0e4                  # off-block px sentinel -> zero bilinear weight


def _f16(a):
    return np.ascontiguousarray(np.asarray(a, dtype=np.float32)).astype(
        np.float16)


def _prep_consts():
    pn_x = np.repeat(np.arange(-1, 2, dtype=np.float32), 3)
    pn_y = np.tile(np.arange(-1, 2, dtype=np.float32), 3)
    cxy = np.zeros((18, COLS), np.float32)
    col = np.arange(COLS)
    h = (col // 4) % 4
    w = col % 4
    for n in range(9):
        cxy[n, :] = h + pn_x[n]
        cxy[9 + n, :] = w + pn_y[n]
    p = np.arange(128)
    u_p = ((p % 16) // 4).astype(np.float32).reshape(128, 1)   # i in 0..3
    v_p = (p % 4).astype(np.float32).reshape(128, 1)           # j in 0..3
    return {
        "cxy": _f16(cxy),
        "u_p": -u_p,
        "v_p": -v_p,
        "ident": np.eye(128, dtype=np.float16),
    }


def _prep_weights(inputs):
    W1 = np.asarray(inputs["W1"], np.float32)  # [32, 3, 3, 3]
    W2 = np.asarray(inputs["W2"], np.float32)  # [64, 32, 3, 3]
    W3 = np.asarray(inputs["W3"], np.float32)  # [128, 64, 3, 3]
    Wp = np.asarray(inputs["Wp"], np.float32)  # [18, 128, 3, 3]
    Wd = np.asarray(inputs["Wd"], np.float32)  # [128, 128, 3, 3]
    Wc = np.asarray(inputs["Wc"], np.float32)  # [10, 128]

    l1 = np.zeros((27, 32), np.float32)        # rows r = t*3 + c
    for t in range(9):
        for c in range(3):
            l1[t * 3 + c] = W1[:, c, t // 3, t % 3]
    l2 = np.zeros((96, 3 * 64), np.float32)    # [tx*32+c, ty-major cols]
    for ty in range(3):
        for tx in range(3):
            l2[tx * 32:(tx + 1) * 32, ty * 64:(ty + 1) * 64] = \
                W2[:, :, ty, tx].T
    l3 = np.zeros((64, 9 * 128), np.float32)   # [c, t-major cols]
    for t in range(9):
        l3[:, t * 128:(t + 1) * 128] = W3[:, :, t // 3, t % 3].T
    lp = np.zeros((128, 9 * 18), np.float32)
    for t in range(9):
        lp[:, t * 18:(t + 1) * 18] = Wp[:, :, t // 3, t % 3].T
    ld = np.zeros((128, 9 * 128), np.float32)  # n = kx*3 + ky
    for n in range(9):
        ld[:, n * 128:(n + 1) * 128] = Wd[:, :, n // 3, n % 3].T
    out = {
        "l1": _f16(l1), "l2": _f16(l2), "l3": _f16(l3),
        "lp": _f16(lp), "ld": _f16(ld),
        "lc": np.ascontiguousarray(Wc.T, np.float32),
        "bp": np.asarray(inputs["bp"], np.float32).reshape(18, 1),
        "bc": np.asarray(inputs["bc"], np.float32).reshape(10, 1),
    }
    for i in (1, 2, 3, 4):
        out[f"g{i}"] = np.asarray(inputs[f"g{i}"], np.float32).reshape(-1, 1)
        out[f"b{i}"] = np.asarray(inputs[f"b{i}"], np.float32).reshape(-1, 1)
    return out


def _prep_x_shard(x_shard):
    """[BS, 3, 4, 4] -> 3x3/pad1 im2col [27, (b, y, x)] fp16, rows t*3+c."""
    x = np.asarray(x_shard, np.float32)
    xp = np.zeros((BS, 3, 6, 6), np.float32)
    xp[:, :, 1:5, 1:5] = x
    out = np.zeros((27, BS, 4, 4), np.float32)
    for t in range(9):
        ty, tx = t // 3, t % 3
        for c in range(3):
            out[t * 3 + c] = xp[:, c, ty:ty + 4, tx:tx + 4]
    return _f16(out.reshape(27, COLS))


def split_waits(nc, maxw=1):
    """This walrus build rejects >1 semaphore wait per instruction; hoist
    extra waits onto NOPs inserted immediately before the instruction."""
    n_split = 0
    for bb in nc.main_func.blocks:
        new_insts = []
        for ins in bb.instructions:
            si = ins.sync_info
            if si is not None and si.on_wait and len(si.on_wait) > maxw:
                waits = list(si.on_wait)
                head, keep = waits[:-maxw], waits[-maxw:]
                for i in range(0, len(head), maxw):
                    nop = mybir.InstNoOp(
                        name=f"I-waitsplit-{nc.next_id()}",
                        engine=ins.engine,
                        sync_info=mybir.SyncInfo(
                            on_wait=list(head[i:i + maxw]), on_update=[]),
                        bass_nofuse=True,
                    )
                    nc.register_instruction(nop)
                    new_insts.append(nop)
                si.on_wait = keep
                n_split += 1
            new_insts.append(ins)
        bb.instructions[:] = new_insts
    return n_split


def _tap_geom(t):
    ty, tx = t // 3, t % 3
    dy, dx = ty - 1, tx - 1
    y0, y1 = max(0, -dy), min(4, 4 - dy)
    x0, x1 = max(0, -dx), min(4, 4 - dx)
    return dy, dx, y0, y1, x0, x1


def build_program():
    nc = bass.Bass()

    xim_d = nc.declare_dram_parameter("xim", [27, COLS], F16, isOutput=False)
    cxy_d = nc.declare_dram_parameter("cxy", [18, COLS], F16, isOutput=False)
    u_d = nc.declare_dram_parameter("u_p", [128, 1], F32, isOutput=False)
    v_d = nc.declare_dram_parameter("v_p", [128, 1], F32, isOutput=False)
    id_d = nc.declare_dram_parameter("ident", [128, 128], F16, isOutput=False)
    l1_d = nc.declare_dram_parameter("l1", [27, 32], F16, isOutput=False)
    l2_d = nc.declare_dram_parameter("l2", [96, 192], F16, isOutput=False)
    l3_d = nc.declare_dram_parameter("l3", [64, 1152], F16, isOutput=False)
    lp_d = nc.declare_dram_parameter("lp", [128, 162], F16, isOutput=False)
    ld_d = nc.declare_dram_parameter("ld", [128, 1152], F16, isOutput=False)
    lc_d = nc.declare_dram_parameter("lc", [128, 10], F32, isOutput=False)
    bp_d = nc.declare_dram_parameter("bp", [18, 1], F32, isOutput=False)
    bc_d = nc.declare_dram_parameter("bc", [10, 1], F32, isOutput=False)
    gb_d = {}
    for i, c in ((1, 32), (2, 64), (3, 128), (4, 128)):
        gb_d[i] = (
            nc.declare_dram_parameter(f"g{i}", [c, 1], F32, isOutput=False),
            nc.declare_dram_parameter(f"b{i}", [c, 1], F32, isOutput=False),
        )
    y_d = nc.declare_dram_parameter("y", [10, BS], F32, isOutput=True)

    PC = 36 * BS   # padded cols per channel: (b, 6, 6)

    def interior(t, c):
        """[c, (b,4,4)]-shaped interior view of a padded [*, PC] tile."""
        return t[0:c, :].rearrange("c (b y x) -> c b y x", y=6, x=6)[
            :, :, 1:5, 1:5]

    with tile.TileContext(nc, num_cores=NCORES) as tc:
        with (
            tc.tile_pool(name="persist", bufs=1) as pp,
            tc.tile_pool(name="dram", bufs=1, space="DRAM") as pdram,
        ):
            def load(dram, shape, dtype):
                t = pp.tile(shape, dtype, tag=dram.name, name=dram.name)
                nc.sync.dma_start(t[:], dram[:])
                return t

            cxy = load(cxy_d, [18, COLS], F16)
            u_p = load(u_d, [128, 1], F32)
            v_p = load(v_d, [128, 1], F32)
            ident = load(id_d, [128, 128], F16)
            l1 = load(l1_d, [27, 32], F16)
            l2a = load(l2_d, [96, 192], F16)
            l3a = load(l3_d, [64, 1152], F16)
            lpa = load(lp_d, [128, 162], F16)
            lda = load(ld_d, [128, 1152], F16)
            l2 = [l2a[:, ty * 64:(ty + 1) * 64] for ty in range(3)]
            l3 = [l3a[:, t * 128:(t + 1) * 128] for t in range(9)]
            lp = [lpa[:, t * 18:(t + 1) * 18] for t in range(9)]
            ld = [lda[:, n * 128:(n + 1) * 128] for n in range(9)]
            lc = load(lc_d, [128, 10], F32)
            bp = load(bp_d, [18, 1], F32)
            bc = load(bc_d, [10, 1], F32)
            gb = {i: (load(g, [g.shape[0], 1], F32),
                      load(b, [b.shape[0], 1], F32))
                  for i, (g, b) in gb_d.items()}

            x3p = pp.tile([128, PC], F16)
            x3u = pp.tile([128, COLS], F16)
            z4 = pp.tile([128, COLS], F16)
            scratch = pp.tile([128, COLS], mybir.dt.float8e4)
            stats = pp.tile([128, 2], F32)
            stats8 = pp.tile([128, 16], F32)   # cols 0:8 sums, 8:16 sumsq
            mvec = pp.tile([128, 1], F32)
            vvec = pp.tile([128, 1], F32)
            tvec = pp.tile([128, 1], F32)
            sc_t = pp.tile([128, 1], F32)
            sh_t = pp.tile([128, 1], F32)

            # warm up the collective path early (absorbs first-call init)
            cc_w_in = pdram.tile([8, 2], F32, name="cc_w_in")
            cc_w_out = pdram.tile([8, 2], F32, addr_space="Shared",
                                  name="cc_w_out")
            warm_sb = pp.tile([8, 2], F32)
            nc.vector.memset(warm_sb[:], 0.0)
            nc.sync.dma_start(cc_w_in[:], warm_sb[:])
            nc.gpsimd.collective_compute(
                "AllReduce", AL.add, replica_groups=[list(range(NCORES))],
                ins=[cc_w_in[:]], outs=[cc_w_out[:]])

            px_rep = pp.tile([128, (NCH // 2) * SCH], F16)
            py_rep = pp.tile([128, (NCH // 2) * SCH], F16)
            nc.vector.memset(px_rep[:], BIG)
            nc.vector.memset(py_rep[:], BIG)

            cc_ins = [pdram.tile([128, 2], F32, name=f"cc_in_{i}")
                      for i in range(4)]
            cc_outs = [pdram.tile([128, 2], F32, addr_space="Shared",
                                  name=f"cc_out_{i}") for i in range(4)]
            pxflat = pdram.tile([S], F16)
            pyflat = pdram.tile([S], F16)

            def chunk_stats(data_ap, c, ch, flat_scr):
                """accumulate per-chunk sum/sumsq partials into stats8."""
                nc.vector.tensor_reduce(
                    out=stats8[0:c, ch:ch + 1], in_=data_ap, op=AL.add,
                    axis=(mybir.AxisListType.X if len(data_ap.shape) == 2
                          else mybir.AxisListType.XYZ))
                nc.scalar.activation(
                    out=flat_scr, in_=data_ap, func=AF.Square,
                    accum_out=stats8[0:c, 8 + ch:9 + ch])

            def stats_and_bn(c, layer, apply_now=True, data_ap=None):
                nc.vector.tensor_reduce(
                    out=stats[0:c, 0:1], in_=stats8[0:c, 0:8], op=AL.add,
                    axis=mybir.AxisListType.X)
                nc.vector.tensor_reduce(
                    out=stats[0:c, 1:2], in_=stats8[0:c, 8:16], op=AL.add,
                    axis=mybir.AxisListType.X)
                cc_in, cc_out = cc_ins[layer - 1], cc_outs[layer - 1]
                nc.sync.dma_start(cc_in[0:c, :], stats[0:c, :])
                nc.gpsimd.collective_compute(
                    "AllReduce", AL.add,
                    replica_groups=[list(range(NCORES))],
                    ins=[cc_in[0:c, :]], outs=[cc_out[0:c, :]])
                nc.sync.dma_start(stats[0:c, :], cc_out[0:c, :])
                g_ap, b_ap = gb[layer]
                nc.vector.tensor_scalar(
                    out=mvec[0:c, :], in0=stats[0:c, 0:1],
                    scalar1=1.0 / CNT, scalar2=None, op0=AL.mult)
                nc.vector.tensor_scalar(
                    out=vvec[0:c, :], in0=stats[0:c, 1:2],
                    scalar1=1.0 / CNT, scalar2=EPS, op0=AL.mult, op1=AL.add)
                nc.vector.tensor_tensor(
                    out=tvec[0:c, :], in0=mvec[0:c, :], in1=mvec[0:c, :],
                    op=AL.mult)
                nc.vector.tensor_tensor(
                    out=vvec[0:c, :], in0=vvec[0:c, :], in1=tvec[0:c, :],
                    op=AL.subtract)
                nc.scalar.activation(
                    out=tvec[0:c, :], in_=vvec[0:c, :], func=AF.Sqrt)
                nc.vector.reciprocal(out=tvec[0:c, :], in_=tvec[0:c, :])
                nc.vector.tensor_tensor(
                    out=sc_t[0:c, :], in0=tvec[0:c, :], in1=g_ap[0:c, :],
                    op=AL.mult)
                nc.vector.tensor_tensor(
                    out=tvec[0:c, :], in0=mvec[0:c, :], in1=sc_t[0:c, :],
                    op=AL.mult)
                nc.vector.tensor_tensor(
                    out=sh_t[0:c, :], in0=b_ap[0:c, :], in1=tvec[0:c, :],
                    op=AL.subtract)
                if apply_now and data_ap is not None:
                    nc.vector.tensor_scalar(
                        out=data_ap, in0=data_ap,
                        scalar1=sc_t[0:c, :], scalar2=sh_t[0:c, :],
                        op0=AL.mult, op1=AL.add)

            # ================= conv layers 1-3 + offset conv =============
            with (
                tc.tile_pool(name="ph123", bufs=1) as p123,
                tc.tile_pool(name="pcv", bufs=2, space="PSUM") as pconv,
            ):
                xim = p123.tile([27, COLS], F16)
                nc.sync.dma_start(xim[:], xim_d[:])
                rhs2 = p123.tile([96, 6144], F16)
                x2p = p123.tile([64, PC], F16)
                nc.scalar.memzero(rhs2[:])
                nc.vector.memset(x2p[:], 0.0)
                nc.scalar.memzero(x3p[:])

                x2i = interior(x2p, 64)
                x3i = interior(x3p, 128)

                # conv1 evicts straight into the conv2 x-im2col operand:
                # rhs2[(tx,c), (b, yp, x)] = relu(conv1)[c, b, yp-1, x+tx-1]
                r2v = rhs2[:].rearrange("r (b y x) -> r b y x", y=6, x=4)
                for ch in range(8):
                    ps = pconv.tile([128, 512], F32, tag="cps")
                    nc.tensor.matmul(
                        ps[0:32, :], l1[:, :],
                        xim[:, ch * 512:(ch + 1) * 512],
                        start=True, stop=True)
                    psv = ps[0:32, :].rearrange(
                        "c (b y x) -> c b y x", y=4, x=4)
                    cs = slice(ch * 32, (ch + 1) * 32)
                    nc.scalar.activation(
                        out=r2v[0:32, cs, 1:5, 1:4], in_=psv[:, :, :, 0:3],
                        func=AF.Relu)
                    nc.scalar.activation(
                        out=r2v[32:64, cs, 1:5, :], in_=psv[:, :, :, :],
                        func=AF.Relu)
                    nc.scalar.activation(
                        out=r2v[64:96, cs, 1:5, 0:3], in_=psv[:, :, :, 1:4],
                        func=AF.Relu)
                    chunk_stats(
                        r2v[32:64, cs, 1:5, :], 32, ch,
                        scratch[0:32, ch * 512:(ch + 1) * 512].rearrange(
                            "c (b y x) -> c b y x", y=4, x=4))
                stats_and_bn(32, 1, apply_now=False)
                nc.vector.tensor_scalar(
                    out=r2v[0:32, :, 1:5, 1:4], in0=r2v[0:32, :, 1:5, 1:4],
                    scalar1=sc_t[0:32, :], scalar2=sh_t[0:32, :],
                    op0=AL.mult, op1=AL.add)
                nc.vector.tensor_scalar(
                    out=r2v[32:64, :, 1:5, :], in0=r2v[32:64, :, 1:5, :],
                    scalar1=sc_t[0:32, :], scalar2=sh_t[0:32, :],
                    op0=AL.mult, op1=AL.add)
                nc.vector.tensor_scalar(
                    out=r2v[64:96, :, 1:5, 0:3], in0=r2v[64:96, :, 1:5, 0:3],
                    scalar1=sc_t[0:32, :], scalar2=sh_t[0:32, :],
                    op0=AL.mult, op1=AL.add)
                for ch in range(8):
                    ps = pconv.tile([128, 512], F32, tag="cps")
                    for ty in range(3):
                        nc.tensor.matmul(
                            ps[0:64, :],
                            l2[ty],
                            r2v[:, ch * 32:(ch + 1) * 32, ty:ty + 4, :],
                            start=(ty == 0), stop=(ty == 2))
                    nc.scalar.activation(
                        out=x2i[:, ch * 32:(ch + 1) * 32, :, :],
                        in_=ps[0:64, :].rearrange(
                            "c (b y x) -> c b y x", y=4, x=4),
                        func=AF.Relu)
                    chunk_stats(
                        x2i[:, ch * 32:(ch + 1) * 32, :, :], 64, ch,
                        scratch[0:64, ch * 512:(ch + 1) * 512].rearrange(
                            "c (b y x) -> c b y x", y=4, x=4))
                stats_and_bn(64, 2, data_ap=x2i)

                x2pv = x2p[:].rearrange("c (b y x) -> c b y x", y=6, x=6)
                for ch in range(8):
                    ps = pconv.tile([128, 512], F32, tag="cps")
                    for t in range(9):
                        ty, tx = t // 3, t % 3
                        nc.tensor.matmul(
                            ps[:, :],
                            l3[t],
                            x2pv[:, ch * 32:(ch + 1) * 32,
                                 ty:ty + 4, tx:tx + 4],
                            start=(t == 0), stop=(t == 8))
                    nc.scalar.activation(
                        out=x3i[:, ch * 32:(ch + 1) * 32, :, :],
                        in_=ps[:, :].rearrange(
                            "c (b y x) -> c b y x", y=4, x=4),
                        func=AF.Relu)
                    nc.scalar.activation(
                        out=x3u[:, ch * 512:(ch + 1) * 512], in_=ps[:, :],
                        func=AF.Relu)
                    chunk_stats(
                        x3u[:, ch * 512:(ch + 1) * 512], 128, ch,
                        scratch[:, ch * 512:(ch + 1) * 512])
                stats_and_bn(128, 3, data_ap=x3i)
                nc.vector.tensor_scalar(
                    out=x3u[:, :], in0=x3u[:, :],
                    scalar1=sc_t[:, :], scalar2=sh_t[:, :],
                    op0=AL.mult, op1=AL.add)

            with tc.tile_pool(name="poff", bufs=1) as poff:
                pxy = poff.tile([18, COLS], F16)
                x3pv = x3p[:].rearrange("c (b y x) -> c b y x", y=6, x=6)
                with tc.tile_pool(name="pcv2", bufs=2, space="PSUM") as pcv2:
                    for ch in range(8):
                        ps = pcv2.tile([128, 512], F32, tag="ops")
                        for t in range(9):
                            ty, tx = t // 3, t % 3
                            nc.tensor.matmul(
                                ps[0:18, :],
                                lp[t],
                                x3pv[:, ch * 32:(ch + 1) * 32,
                                     ty:ty + 4, tx:tx + 4],
                                start=(t == 0), stop=(t == 8))
                        nc.scalar.activation(
                            out=pxy[:, ch * 512:(ch + 1) * 512],
                            in_=ps[0:18, :],
                            func=AF.Identity, bias=bp[:, :])
                nc.vector.tensor_tensor(
                    out=pxy[:, :], in0=pxy[:, :], in1=cxy[:, :], op=AL.add)
                nc.sync.dma_start(
                    pxflat[:].rearrange("(b n hw) -> n b hw", n=9, hw=16),
                    pxy[0:9, :].rearrange("n (b hw) -> n b hw", hw=16))
                nc.scalar.dma_start(
                    pyflat[:].rearrange("(b n hw) -> n b hw", n=9, hw=16),
                    pxy[9:18, :].rearrange("n (b hw) -> n b hw", hw=16))

            # ================= deformable sampling + deform conv =========
            NQ2 = 2
            CH2 = NCH // NQ2       # 16 chunks per half
            with (
                tc.tile_pool(name="dfm1", bufs=1) as pdf1,
                tc.tile_pool(name="dfm2", bufs=3) as pdf2,
                tc.tile_pool(name="dfm3", bufs=4) as pdf3,
                tc.tile_pool(name="ptr", bufs=1, space="PSUM") as ptr,
                tc.tile_pool(name="psmp", bufs=2, space="PSUM") as psamp,
                tc.tile_pool(name="pdef", bufs=1, space="PSUM") as pdef,
            ):
                pxf_v = pxflat[:].rearrange("(k s) -> k s", s=SCH)
                pyf_v = pyflat[:].rearrange("(k s) -> k s", s=SCH)
                pxr_v = px_rep[:].rearrange("p (k s) -> p k s", s=SCH)
                pyr_v = py_rep[:].rearrange("p (k s) -> p k s", s=SCH)
                for q in range(NQ2):
                    k0 = q * CH2
                    for g in range(8):
                        nc.sync.dma_start(
                            pxr_v[g * 16:(g + 1) * 16, :,
                                  g * 144:(g + 1) * 144],
                            pxf_v[k0:k0 + CH2, g * 144:(g + 1) * 144]
                            .unsqueeze(0).broadcast_to([16, CH2, 144]))
                        nc.sync.dma_start(
                            pyr_v[g * 16:(g + 1) * 16, :,
                                  g * 144:(g + 1) * 144],
                            pyf_v[k0:k0 + CH2, g * 144:(g + 1) * 144]
                            .unsqueeze(0).broadcast_to([16, CH2, 144]))
                    for kk in range(CH2):
                        k = k0 + kk
                        sl = slice(kk * SCH, (kk + 1) * SCH)
                        wxy = pdf2.tile([128, 2 * SCH], F16, tag="wxy")
                        mT = pdf3.tile([128, SCH], F16, tag="mT")
                        # |px - i| on ACT (bias = -i), then min/sub + product
                        nc.scalar.activation(
                            out=wxy[:, 0:SCH], in_=px_rep[:, sl],
                            func=AF.Abs, bias=u_p[:, :])
                        nc.scalar.activation(
                            out=wxy[:, SCH:2 * SCH], in_=py_rep[:, sl],
                            func=AF.Abs, bias=v_p[:, :])
                        nc.vector.tensor_scalar(
                            out=wxy[:], in0=wxy[:], scalar1=1.0, scalar2=1.0,
                            op0=AL.min, op1=AL.subtract)
                        nc.vector.tensor_tensor(
                            out=mT[:], in0=wxy[:, 0:SCH],
                            in1=wxy[:, SCH:2 * SCH], op=AL.mult)
                        tps = ptr.tile([128, 128], F16, tag="tps")
                        nc.tensor.transpose(
                            tps[:], x3u[:, k * 128:(k + 1) * 128], ident[:])
                        xT = pdf3.tile([128, 128], F16, tag="xT")
                        nc.vector.tensor_copy(xT[:], tps[:])
                        if kk % 4 == 0:
                            xoff = pdf2.tile([128, 4 * SCH], F16, tag="xoff")
                        sp = psamp.tile([128, 1152], F32, tag="sp")
                        for j, (c0, c1) in enumerate(
                                ((0, 512), (512, 1024), (1024, 1152))):
                            nc.tensor.matmul(
                                sp[:, c0:c1], xT[:], mT[:, c0:c1],
                                start=True, stop=True)
                        nc.vector.tensor_copy(
                            xoff[:, (kk % 4) * SCH:(kk % 4 + 1) * SCH],
                            sp[:, 0:1152])
                        if kk % 4 == 3:
                            gi = k // 4
                            pd = pdef.tile([128, 512], F32, tag="pd")
                            xov = xoff[:].rearrange(
                                "c (b n hw) -> c b n hw", n=9, hw=16)
                            for n in range(9):
                                nc.tensor.matmul(
                                    pd[:], ld[n], xov[:, :, n, :],
                                    start=(n == 0), stop=(n == 8))
                            nc.scalar.activation(
                                out=z4[:, gi * 512:(gi + 1) * 512],
                                in_=pd[:], func=AF.Relu)
                            chunk_stats(
                                z4[:, gi * 512:(gi + 1) * 512], 128, gi,
                                scratch[:, gi * 512:(gi + 1) * 512])

            # ================= BN4 stats + mean + fc =====================
            stats_and_bn(128, 4, apply_now=False)
            with (
                tc.tile_pool(name="fc", bufs=1) as pfc,
                tc.tile_pool(name="pfc", bufs=1, space="PSUM") as ppfc,
            ):
                feats = pfc.tile([128, BS], F32)
                nc.vector.reduce_sum(
                    out=feats[:, :],
                    in_=z4[:].rearrange("c (b hw) -> c b hw", hw=16),
                    axis=mybir.AxisListType.X)
                nc.vector.tensor_scalar(
                    out=sc_t[:, :], in0=sc_t[:, :], scalar1=1.0 / 16,
                    scalar2=None, op0=AL.mult)
                nc.vector.tensor_scalar(
                    out=feats[:, :], in0=feats[:, :],
                    scalar1=sc_t[:, :], scalar2=sh_t[:, :],
                    op0=AL.mult, op1=AL.add)
                pf = ppfc.tile([10, BS], F32)
                nc.tensor.matmul(pf[:], lc[:, :], feats[:, :],
                                 start=True, stop=True)
                out_sb = pfc.tile([10, BS], F32)
                nc.vector.tensor_scalar(
                    out=out_sb[:], in0=pf[:], scalar1=bc[:, :], scalar2=None,
                    op0=AL.add)
                nc.sync.dma_start(y_d[:], out_sb[:])

    split_waits(nc)
    return nc


_PROGRAM = None


def run(inputs, trace=False, **kw):
    global _PROGRAM
    if _PROGRAM is None:
        _PROGRAM = build_program()
    nc = _PROGRAM

    shared = {**_prep_consts(), **_prep_weights(inputs)}
    x = np.asarray(inputs["x"], np.float32)
    in_maps = []
    for i in range(NCORES):
        m = dict(shared)
        m["xim"] = _prep_x_shard(x[i * BS:(i + 1) * BS])
        in_maps.append(m)

    res = run_bass_kernel_spmd(nc, in_maps, list(range(NCORES)),
                               trace=trace, **kw)
    out = np.concatenate(
        [np.asarray(res.results[i]["y"]).T for i in range(NCORES)], axis=0)
    return np.ascontiguousarray(out, np.float32), res


def kernel(**inputs):
    out, _ = run(inputs)
    return out


if __name__ == "__main__":
    d = np.load("/root/problem/ref_data.npz")
    inputs = {k: d[k] for k in d.files if k != "expected"}
    got = kernel(**inputs)
    exp = d["expected"]
    err = np.abs(got - exp).max()
    print("absmax err:", err, "relmax:", err / np.abs(exp).max())

